# revision 43
# baseline (speedup 1.0000x reference)
"""Distributional (Gaussian-KL) attention on 8 TRN2 NeuronCores.

Math: for each head, the KL-based score decomposes as
    kl[q,k] = sum_d (Qm-Km)^2/(2Kv) + 0.5*(Qv/Kv - log(Qv/Kv) - 1)
            = Fq[q] . Fk[k] + r[k] + c[q]
with  Fq = [Qm^2+Qv ; -Qm],  Fk = [1/(2Kv) ; Km/Kv],
      r  = 0.5*sum_d (Km^2/Kv + log Kv),
and c[q] only shifts softmax logits per-row (drops out of softmax over k).
scores = -kl/sqrt(Dh); kl >= 0 so exp(scores) needs no max-shift.

Sharding: head-parallel front (core c owns heads {2c,2c+1} == feature
columns [128c,128c+128) of every Q/K/V projection), then SPLIT-ROLE
output: cores 0-3 compute out_mu columns [256c,256c+256), cores 4-7
compute out_var columns [256(c-4),...). The role is purely data-driven
(per-core wo/bias inputs), so the program stays SPMD. A single AllToAll
delivers to each core the full O-half it needs (chunk j of core i's send
buffer = i's O_mu block for j<4, else its O_var block): one collective,
half the inbound bytes of a full AllGather of both halves. Every core
runs BOTH epilogues (identity for mu, softplus for var) on its projected
block; the host keeps the meaningful one per core.

Host-side prep (inside kernel(), numpy only): weights/inputs pre-cast to
bf16 and pre-transposed/tiled into PE-friendly layouts - no on-device
casts or input transposes.

Precision: all matmuls bf16 except the r_k path (fp32; r is O(100+) and
added to logits as a bf16 hi+lo pair via K=2 rank-2 matmuls against a
0.5-valued lhsT, since duplicate rows avoid illegal partition bases).

ACT table discipline (a table swap costs ~1.28us): sigmoid x3 -> one
wide Ln (+ t_lg Ln) -> exp (attention + output-softplus numerator; a
dep-free warm op prefetches the table during the score matmuls) -> one
wide final Ln.  softplus(x)=ln(sigmoid(-x)) up front, ln(1+exp(x)) at
the output so it reuses the resident exp table.  1/Kv runs on DVE in
parallel with ACT.
"""

import numpy as np

import concourse.bass as bass
import concourse.mybir as mybir
import concourse.tile as tile
from concourse import bacc
from concourse.masks import make_identity
from concourse.bass_utils import run_bass_kernel_spmd

F32 = mybir.dt.float32
BF16 = mybir.dt.bfloat16
AF = mybir.ActivationFunctionType
ALU = mybir.AluOpType
AX = mybir.AxisListType

H, B, L, D = 16, 1, 256, 1024
Dh = D // H          # 64
NCORES = 8
CB = D // NCORES     # 128 feature columns per core (2 heads)
P = 128
LT = L // P          # 2 row tiles of the sequence
KT = D // P          # 8 contraction tiles
NW = 8               # weights: qv kv vv qm km vm | out-half col-tiles 0,1

TRACE = False
TRACE_KWARGS = {}
LAST_RESULT = None

_prog_cache = {}


def ts(i, size):
    return slice(i * size, (i + 1) * size)


def build_program():
    nc = bacc.Bacc("TRN2", target_bir_lowering=False, debug=False,
                   num_devices=NCORES)

    # xcat[p, s, kt, l] = x_s[l, kt*128+p]; s: 0=var, 1=mu
    xcat_d = nc.dram_tensor("xcat", [P, 2, KT, L], BF16, kind="ExternalInput")
    # wcat[p, w, kt, m] = w[kt*128+p, m] (slots 6,7: core's out-half wo)
    wcat_d = nc.dram_tensor("wcat", [P, NW, KT, CB], BF16,
                            kind="ExternalInput")
    b_d = nc.dram_tensor("biases", [CB, 10], F32, kind="ExternalInput")
    # out_a: identity epilogue (mu), out_b: softplus epilogue (var);
    # rows [0:128) = col-tile 0, [128:256) = col-tile 1 of the core's
    # 256-wide output block. Host keeps a for cores 0-3, b for 4-7.
    out_a_d = nc.dram_tensor("out_a", [2 * CB, L], F32,
                             kind="ExternalOutput")
    out_b_d = nc.dram_tensor("out_b", [2 * CB, L], F32,
                             kind="ExternalOutput")

    with tile.TileContext(nc) as tc:
        _build(nc, tc, xcat_d, wcat_d, b_d, out_a_d, out_b_d)
    nc.compile()
    return nc


def _build(nc, tc, xcat_d, wcat_d, b_d, out_a_d, out_b_d):
    from contextlib import ExitStack
    ctx = ExitStack()
    with ctx:
        const = ctx.enter_context(tc.tile_pool(name="const", bufs=1))
        persist = ctx.enter_context(tc.tile_pool(name="persist", bufs=1))
        stage = ctx.enter_context(tc.tile_pool(name="stage", bufs=3))
        feat = ctx.enter_context(tc.tile_pool(name="feat", bufs=1))
        attnp = ctx.enter_context(tc.tile_pool(name="attnp", bufs=2))
        ps_proj = ctx.enter_context(
            tc.tile_pool(name="ps_proj", bufs=2, space="PSUM"))
        ps_tr = ctx.enter_context(
            tc.tile_pool(name="ps_tr", bufs=2, space="PSUM"))
        ps_s = ctx.enter_context(
            tc.tile_pool(name="ps_s", bufs=2, space="PSUM"))
        ps_small = ctx.enter_context(
            tc.tile_pool(name="ps_small", bufs=1, space="PSUM"))
        dram = ctx.enter_context(tc.tile_pool(name="dram", bufs=1,
                                              space="DRAM"))

        # ---------------- inputs: clean bf16 DMAs, finest useful order --
        # First projection (wq_var @ x_var) is the PE critical path start:
        # split its two operands into half-kt chunks so it can begin as
        # soon as the first halves land.
        x_sb = persist.tile([P, 2, KT, L], BF16, tag="x_sb", name="x_sb")
        w_sb = persist.tile([P, NW, KT, CB], BF16, tag="w_sb", name="w_sb")
        QK = KT // 4
        nc.sync.dma_start(x_sb[:, 0, 0:QK], xcat_d.ap()[:, 0, 0:QK])
        nc.scalar.dma_start(w_sb[:, 0, 0:QK], wcat_d.ap()[:, 0, 0:QK])
        nc.sync.dma_start(x_sb[:, 0, QK:KT], xcat_d.ap()[:, 0, QK:KT])
        nc.scalar.dma_start(w_sb[:, 0, QK:KT], wcat_d.ap()[:, 0, QK:KT])
        for wi in range(1, 3):                               # k,v_var
            nc.sync.dma_start(w_sb[:, wi:wi + 1], wcat_d.ap()[:, wi:wi + 1])
        nc.sync.dma_start(x_sb[:, 1], xcat_d.ap()[:, 1])     # mu
        for wi in range(3, 6):                               # qkv_mu
            nc.sync.dma_start(w_sb[:, wi:wi + 1], wcat_d.ap()[:, wi:wi + 1])
        nc.sync.dma_start(w_sb[:, 6:8], wcat_d.ap()[:, 6:8])  # out-half wo

        # ---------------- constants -----------------------------------
        ident_b = const.tile([P, P], BF16, tag="ident_b", name="ident_b")
        make_identity(nc, ident_b)
        half2 = const.tile([2, P], BF16, tag="half2", name="half2")
        nc.vector.memset(half2, 0.5)
        # ind_h[p, :] = 0.5 if p in head h else 0 (both cols; the fp32 r
        # matmul then yields r_h duplicated on partitions {0,1})
        ind_h = []
        for h in range(2):
            ih = const.tile([P, 2], F32, tag=f"ind{h}", name=f"ind{h}")
            nc.vector.memset(ih, 0.0)
            nc.vector.memset(ih[ts(h, Dh), :], 0.5)
            ind_h.append(ih)

        B_ORDER = ["bq_mu", "bq_var", "bk_mu", "bk_var", "bv_mu", "bv_var",
                   "bo_mu", "bo_var", "bout0", "bout1"]
        bcat = const.tile([CB, 10], F32, tag="bcat", name="bcat")
        nc.sync.dma_start(bcat, b_d.ap())
        nbcat = const.tile([CB, 10], F32, tag="nbcat", name="nbcat")
        nc.vector.tensor_scalar_mul(nbcat, bcat, -1.0)
        bias = {n: bcat[:, i:i + 1] for i, n in enumerate(B_ORDER)}
        nbias = {n: nbcat[:, i:i + 1] for i, n in enumerate(B_ORDER)}

        # warm the sigmoid table while DMAs run
        warm_sg = stage.tile([1, 1], F32, tag="warm_sg", name="warm_sg",
                             bufs=1)
        nc.scalar.activation(warm_sg, ident_b[0:1, 0:1], AF.Sigmoid)

        # ---------------- projections (feature-major [CB, L], bf16) ----
        def project(wi, si):
            ps = ps_proj.tile([P, L], F32, tag="proj", name="proj")
            for kt in range(KT):
                nc.tensor.matmul(ps, w_sb[:, wi, kt, :], x_sb[:, si, kt, :],
                                 start=(kt == 0), stop=(kt == KT - 1))
            return ps

        # var side first: 3 sigmoids (one table residency), then one wide
        # Ln over all three at once (forces sig/sig/sig/ln queue order).
        sg_all = feat.tile([P, 3 * L], F32, tag="sg_all", name="sg_all")
        ps_qv = project(0, 0)
        nc.scalar.activation(sg_all[:, 0 * L:1 * L], ps_qv, AF.Sigmoid,
                             scale=-1.0, bias=nbias["bq_var"])
        ps_kv = project(1, 0)
        nc.scalar.activation(sg_all[:, 1 * L:2 * L], ps_kv, AF.Sigmoid,
                             scale=-1.0, bias=nbias["bk_var"])
        ps_vv = project(2, 0)
        nc.scalar.activation(sg_all[:, 2 * L:3 * L], ps_vv, AF.Sigmoid,
                             scale=-1.0, bias=nbias["bv_var"])

        # nsp_* = ln(sigmoid(-(x+b))) = -softplus(x+b)
        nsp_all = feat.tile([P, 3 * L], F32, tag="nsp_all", name="nsp_all")
        nc.scalar.activation(nsp_all, sg_all, AF.Ln)
        nsp_q = nsp_all[:, 0 * L:1 * L]
        nsp_k = nsp_all[:, 1 * L:2 * L]
        nsp_v = nsp_all[:, 2 * L:3 * L]

        t_kv = feat.tile([P, L], F32, tag="t_kv", name="t_kv")
        nc.vector.tensor_scalar_mul(t_kv, nsp_k, -1.0)    # Kv
        t_lg = feat.tile([P, L], F32, tag="t_lg", name="t_lg")
        nc.scalar.activation(t_lg, t_kv, AF.Ln)           # ln Kv

        # mu-side projections (DVE-only extraction)
        ps_qm = project(3, 1)
        t_qm = feat.tile([P, L], F32, tag="t_qm", name="t_qm")
        nc.vector.tensor_scalar_add(t_qm, ps_qm, bias["bq_mu"])
        negqm_bf = feat.tile([P, L], BF16, tag="negqm", name="negqm")
        nc.vector.tensor_scalar_mul(negqm_bf, t_qm, -1.0)
        t_qm2 = feat.tile([P, L], F32, tag="t_qm2", name="t_qm2")
        nc.vector.tensor_mul(t_qm2, t_qm, t_qm)
        ps_km = project(4, 1)
        t_km = feat.tile([P, L], F32, tag="t_km", name="t_km")
        nc.vector.tensor_scalar_add(t_km, ps_km, bias["bk_mu"])
        t_km2 = feat.tile([P, L], F32, tag="t_km2", name="t_km2")
        nc.vector.tensor_mul(t_km2, t_km, t_km)
        ps_vm = project(5, 1)
        vmT = feat.tile([P, L], BF16, tag="vmT", name="vmT")
        nc.vector.tensor_scalar_add(vmT, ps_vm, bias["bv_mu"])

        # derived features
        fq1_bf = feat.tile([P, L], BF16, tag="fq1", name="fq1")
        nc.vector.tensor_sub(fq1_bf, t_qm2, nsp_q)        # Qm^2 + Qv
        vvT = feat.tile([P, L], BF16, tag="vvT", name="vvT")
        nc.vector.tensor_scalar_mul(vvT, nsp_v, -1.0)     # Vv
        # 1/Kv = exp(-ln Kv) on ACT: its table load starts the moment t_lg
        # finishes (it is the first exp op in the queue, and its t_lg
        # input stops the scheduler hoisting it into the sigmoid block),
        # so the attention exps later hit a resident table. This path is
        # the critical chain into the scores' r-term: keep it short.
        t_iv = feat.tile([P, L], F32, tag="t_iv", name="t_iv")
        nc.scalar.activation(t_iv, t_lg, AF.Exp, scale=-1.0)
        fk1_bf = feat.tile([P, L], BF16, tag="fk1", name="fk1")
        nc.vector.tensor_scalar_mul(fk1_bf, t_iv, 0.5)
        kmiv_bf = feat.tile([P, L], BF16, tag="kmiv", name="kmiv")
        nc.vector.tensor_mul(kmiv_bf, t_km, t_iv)
        t_u = feat.tile([P, L], F32, tag="t_u", name="t_u")
        nc.vector.tensor_mul(t_u, t_km2, t_iv)
        t_s = feat.tile([P, L], F32, tag="t_s", name="t_s")
        nc.vector.tensor_add(t_s, t_u, t_lg)

        # V to L-major bf16 via PE transpose
        v_l = {}
        for nm, src in (("vm", vmT), ("vv", vvT)):
            for lk in range(LT):
                pt = ps_tr.tile([P, P], BF16, tag="tr", name="trb")
                nc.tensor.transpose(pt, src[:, ts(lk, P)], ident_b)
                dst = feat.tile([P, P], BF16, tag=f"vl_{nm}_{lk}",
                                name=f"vl_{nm}_{lk}")
                nc.vector.tensor_copy(dst, pt)
                v_l[(nm, lk)] = dst

        # r per head: 0.5 * sum_d (Km^2/Kv + log Kv) (fp32 path). One
        # duplicate-row matmul per head puts r_h on partitions {0,1};
        # the bf16 hi/lo tiles keep both rows identical (no partition
        # shifts) and are added to the logits via K=2 matmuls against a
        # 0.5-valued lhsT (0.5*(v+v) = v).
        r_hi, r_lo = [], []
        for h in range(2):
            prh = ps_small.tile([2, L], F32, tag="r_ps",
                                name=f"r_ps{h}")
            nc.tensor.matmul(prh, ind_h[h], t_s, start=True, stop=True)
            hi2 = feat.tile([2, L], BF16, tag=f"r_hi{h}", name=f"r_hi{h}")
            nc.vector.tensor_copy(hi2, prh)
            lo2f = feat.tile([2, L], F32, tag=f"r_lof{h}", name=f"r_lof{h}")
            nc.vector.tensor_sub(lo2f, prh, hi2)
            lo2 = feat.tile([2, L], BF16, tag=f"r_lo{h}", name=f"r_lo{h}")
            nc.vector.tensor_copy(lo2, lo2f)
            r_hi.append(hi2)
            r_lo.append(lo2)

        # ---------------- attention ------------------------------------
        # Phase 1 streams all score matmuls + softmax (the in-order PE
        # queue must not hold transposes that wait on the ACT/DVE chain);
        # phase 2 then does all the attn transposes + squares.
        a_bfs = {}
        for h in range(2):
            hs = ts(h, Dh)
            for t in range(LT):
                ps_S = ps_s.tile([P, L], F32, tag="scores", name="scores")
                nc.tensor.matmul(ps_S, fq1_bf[hs, ts(t, P)], fk1_bf[hs, :],
                                 start=True, stop=False)
                nc.tensor.matmul(ps_S, negqm_bf[hs, ts(t, P)], kmiv_bf[hs, :],
                                 start=False, stop=False)
                nc.tensor.matmul(ps_S, half2, r_hi[h],
                                 start=False, stop=False)
                nc.tensor.matmul(ps_S, half2, r_lo[h],
                                 start=False, stop=True)
                pexp = attnp.tile([P, L], BF16, tag="pexp", name="pexp")
                den = attnp.tile([P, 1], F32, tag="den", name="den")
                nc.scalar.activation(pexp, ps_S, AF.Exp, bias=0.0,
                                     scale=-0.125, accum_out=den)
                invd = attnp.tile([P, 1], F32, tag="invd", name="invd")
                nc.vector.reciprocal(invd, den)
                a_bf = attnp.tile([P, L], BF16, tag=f"a_bf_{h}_{t}",
                                  name=f"a_bf_{h}_{t}", bufs=1)
                nc.vector.tensor_scalar_mul(a_bf, pexp, invd)
                a_bfs[(h, t)] = a_bf

        attnT = {}   # (h, lk) -> [128 (k within lk), 256 (q)] bf16
        a2T = {}     # squared attention (var path)
        for h in range(2):
            for t in range(LT):
                for lk in range(LT):
                    if (h, lk) not in attnT:
                        attnT[(h, lk)] = feat.tile(
                            [P, L], BF16, tag=f"attnT_{h}_{lk}",
                            name=f"attnT_{h}_{lk}")
                        a2T[(h, lk)] = feat.tile(
                            [P, L], BF16, tag=f"a2T_{h}_{lk}",
                            name=f"a2T_{h}_{lk}")
                    pt = ps_tr.tile([P, P], BF16, tag="tr", name="trb")
                    nc.tensor.transpose(pt, a_bfs[(h, t)][:, ts(lk, P)],
                                        ident_b)
                    nc.vector.tensor_copy(attnT[(h, lk)][:, ts(t, P)], pt)
                    nc.vector.tensor_mul(a2T[(h, lk)][:, ts(t, P)],
                                         attnT[(h, lk)][:, ts(t, P)],
                                         attnT[(h, lk)][:, ts(t, P)])

        # ---------------- PV + single AllToAll --------------------------
        # cc_in chunk j: our O_mu block for j<4, our O_var block for j>=4
        # => core j receives all 8 cores' blocks of the half IT projects.
        cc_in = dram.tile([NCORES * CB, L], BF16, tag="cc_in", name="cc_in")

        def pv_and_stage(vkey, att, oname, row0, eng):
            # per-head pipelining: head h's PSUM->bf16 copy and its
            # broadcast staging DMA (one DIRECT2D writes all 4 chunk
            # copies) overlap head h+1's PV matmuls
            pv = ps_small.tile([P, L], F32, tag="pv", name=f"pv_{oname}",
                               bufs=1)
            o = attnp.tile([P, L], BF16, tag=f"o_{oname}", name=f"o_{oname}")
            dst = cc_in[row0:row0 + 4 * CB, :].rearrange(
                "(b p) l -> p b l", p=P)
            for h in range(2):
                hs = ts(h, Dh)
                for lk in range(LT):
                    nc.tensor.matmul(pv[hs, :],
                                     v_l[(vkey, lk)][:, hs],
                                     att[(h, lk)],
                                     start=(lk == 0), stop=(lk == LT - 1),
                                     tile_position=(0, h * Dh))
                nc.vector.tensor_copy(o[hs, :], pv[hs, :])
                eng.dma_start(
                    dst[hs],
                    o[hs, :].unsqueeze(1).broadcast_to([Dh, 4, L]))

        pv_and_stage("vm", attnT, "mu", 0, nc.scalar)
        pv_and_stage("vv", a2T, "var", 4 * CB, nc.sync)

        cc_out = dram.tile([NCORES * CB, L], BF16, tag="cc_out",
                           name="cc_out")
        nc.gpsimd.collective_compute(
            "AllToAll", ALU.bypass,
            replica_groups=[list(range(NCORES))],
            ins=[cc_in[:].opt()],
            outs=[cc_out[:].opt()],
        )

        # ---------------- output projection (one half, 256 cols) -------
        g = stage.tile([P, KT, L], BF16, tag="gall", name="gall", bufs=1)
        rr = cc_out.rearrange("(c p) m -> p c m", p=P)
        nc.sync.dma_start(g[:, 0:KT // 2], rr[:, 0:KT // 2])
        nc.scalar.dma_start(g[:, KT // 2:KT], rr[:, KT // 2:KT])

        # Both epilogues on both col-tiles; u/w1/res are [P, 2L] wide so
        # the single final Ln depends on BOTH exps (no table thrash).
        u = stage.tile([P, 2 * L], F32, tag="u", name="u", bufs=1)
        w1 = stage.tile([P, 2 * L], F32, tag="w1", name="w1", bufs=1)
        res_sp = stage.tile([P, 2 * L], F32, tag="res_sp", name="res_sp",
                            bufs=1)
        ps_o = []
        for m in range(2):
            ps = ps_proj.tile([P, L], F32, tag="proj", name="proj")
            for kt in range(KT):
                nc.tensor.matmul(ps, w_sb[:, 6 + m, kt, :], g[:, kt, :],
                                 start=(kt == 0), stop=(kt == KT - 1))
            nc.scalar.activation(u[:, ts(m, L)], ps, AF.Exp, scale=1.0,
                                 bias=bias["bout0" if m == 0 else "bout1"])
            ps_o.append(ps)
        nc.vector.tensor_scalar_add(w1, u, 1.0)
        nc.scalar.activation(res_sp, w1, AF.Ln)
        for m in range(2):
            res_id = stage.tile([P, L], F32, tag=f"res_id{m}",
                                name=f"res_id{m}")
            nc.vector.tensor_scalar_add(
                res_id, ps_o[m], bias["bout0" if m == 0 else "bout1"])
            nc.gpsimd.dma_start(out_a_d.ap()[ts(m, CB), :], res_id)
            nc.sync.dma_start(out_b_d.ap()[ts(m, CB), :], res_sp[:, ts(m, L)])


def shard_inputs(inputs):
    """Full inputs -> per-core in_maps (host-side numpy prep only)."""
    f32 = np.float32
    bf16 = mybir.dt.np(BF16)

    def to_pe_tiles(a):      # [1024, n] -> [128, 8, n]
        n = a.shape[1]
        return np.ascontiguousarray(
            a.reshape(KT, P, n).transpose(1, 0, 2))

    xcat = np.empty((P, 2, KT, L), dtype=bf16)
    for si, nm in enumerate(("var", "mu")):
        xt = np.asarray(inputs[nm]).reshape(L, D).astype(f32).T  # [D, L]
        xcat[:, si] = to_pe_tiles(xt.astype(bf16))

    W_ORDER = ["wq_var", "wk_var", "wv_var", "wq_mu", "wk_mu", "wv_mu"]
    B_NAMES = ["bq_mu", "bq_var", "bk_mu", "bk_var", "bv_mu", "bv_var",
               "bo_mu", "bo_var"]
    in_maps = []
    for c in range(NCORES):
        cols = slice(c * CB, (c + 1) * CB)
        wcat = np.empty((P, NW, KT, CB), dtype=bf16)
        for wi, nm in enumerate(W_ORDER):
            w = np.asarray(inputs[nm])[:, cols].astype(f32).astype(bf16)
            wcat[:, wi] = to_pe_tiles(w)
        # out-half role: cores 0-3 -> wo_mu/bo_mu cols [256c, 256c+256),
        # cores 4-7 -> wo_var/bo_var cols [256(c-4), ...)
        if c < 4:
            wo, bo, oc = inputs["wo_mu"], inputs["bo_mu"], 2 * c * CB
        else:
            wo, bo, oc = inputs["wo_var"], inputs["bo_var"], 2 * (c - 4) * CB
        for m in range(2):
            wblk = np.asarray(wo)[:, oc + m * CB: oc + (m + 1) * CB]
            wcat[:, 6 + m] = to_pe_tiles(wblk.astype(f32).astype(bf16))
        bcols = [np.asarray(inputs[n])[cols].astype(f32) for n in B_NAMES]
        bcols.append(np.asarray(bo)[oc: oc + CB].astype(f32))
        bcols.append(np.asarray(bo)[oc + CB: oc + 2 * CB].astype(f32))
        biases = np.ascontiguousarray(np.stack(bcols, axis=1))
        in_maps.append({"xcat": xcat, "wcat": wcat, "biases": biases})
    return in_maps


def kernel(**inputs):
    global LAST_RESULT
    if "prog" not in _prog_cache:
        _prog_cache["prog"] = build_program()
    nc = _prog_cache["prog"]
    in_maps = shard_inputs(inputs)
    res = run_bass_kernel_spmd(nc, in_maps, core_ids=list(range(NCORES)),
                               trace=TRACE, **TRACE_KWARGS)
    LAST_RESULT = res
    mu_blocks = [res.results[c]["out_a"] for c in range(4)]
    var_blocks = [res.results[c]["out_b"] for c in range(4, 8)]
    mu_out = np.concatenate(mu_blocks, axis=0).T.reshape(B, L, D)
    var_out = np.concatenate(var_blocks, axis=0).T.reshape(B, L, D)
    return (np.ascontiguousarray(mu_out.astype(np.float32)),
            np.ascontiguousarray(var_out.astype(np.float32)))


# revision 45
# speedup vs baseline: 1.8007x; 1.8007x over previous
"""Distributional (Gaussian-KL) attention on 8 TRN2 NeuronCores.

Math: for each head, the KL-based score decomposes as
    kl[q,k] = sum_d (Qm-Km)^2/(2Kv) + 0.5*(Qv/Kv - log(Qv/Kv) - 1)
            = Fq[q] . Fk[k] + r[k] + c[q]
with  Fq = [Qm^2+Qv ; -Qm],  Fk = [1/(2Kv) ; Km/Kv],
      r  = 0.5*sum_d (Km^2/Kv + log Kv),
and c[q] only shifts softmax logits per-row (drops out of softmax over k).
scores = -kl/sqrt(Dh); kl >= 0 so exp(scores) needs no max-shift.

Sharding: head-parallel front (core c owns heads {2c,2c+1} == feature
columns [128c,128c+128) of every Q/K/V projection), then SPLIT-ROLE
output: cores 0-3 compute out_mu columns [256c,256c+256), cores 4-7
compute out_var columns [256(c-4),...). The role is purely data-driven
(per-core wo/bias inputs), so the program stays SPMD. A single AllToAll
delivers to each core the full O-half it needs (chunk j of core i's send
buffer = i's O_mu block for j<4, else its O_var block): one collective,
half the inbound bytes of a full AllGather of both halves. Every core
runs BOTH epilogues (identity for mu, softplus for var) on its projected
block; the host keeps the meaningful one per core.

Host-side prep (inside kernel(), numpy only): weights/inputs pre-cast to
bf16 and pre-transposed/tiled into PE-friendly layouts - no on-device
casts or input transposes.

Precision: all matmuls bf16 except the r_k path (fp32; r is O(100+) and
added to logits as a bf16 hi+lo pair via K=2 rank-2 matmuls against a
0.5-valued lhsT, since duplicate rows avoid illegal partition bases).

ACT table discipline (a table swap costs ~1.28us): sigmoid x3 -> one
wide Ln (+ t_lg Ln) -> exp (attention + output-softplus numerator; a
dep-free warm op prefetches the table during the score matmuls) -> one
wide final Ln.  softplus(x)=ln(sigmoid(-x)) up front, ln(1+exp(x)) at
the output so it reuses the resident exp table.  1/Kv runs on DVE in
parallel with ACT.
"""

import numpy as np

import concourse.bass as bass
import concourse.mybir as mybir
import concourse.tile as tile
from concourse import bacc
from concourse.masks import make_identity
from concourse.bass_utils import run_bass_kernel_spmd

F32 = mybir.dt.float32
BF16 = mybir.dt.bfloat16
AF = mybir.ActivationFunctionType
ALU = mybir.AluOpType
AX = mybir.AxisListType

H, B, L, D = 16, 1, 256, 1024
Dh = D // H          # 64
NCORES = 8
CB = D // NCORES     # 128 feature columns per core (2 heads)
P = 128
LT = L // P          # 2 row tiles of the sequence
KT = D // P          # 8 contraction tiles
NW = 8               # weights: qv kv vv qm km vm | out-half col-tiles 0,1

TRACE = False
TRACE_KWARGS = {}
LAST_RESULT = None

_prog_cache = {}


def ts(i, size):
    return slice(i * size, (i + 1) * size)


def build_program():
    nc = bacc.Bacc("TRN2", target_bir_lowering=False, debug=False,
                   num_devices=NCORES)

    # xcat[p, s, kt, l] = x_s[l, kt*128+p]; s: 0=var, 1=mu
    xcat_d = nc.dram_tensor("xcat", [P, 2, KT, L], BF16, kind="ExternalInput")
    # wcat[p, w, kt, m] = w[kt*128+p, m] (slots 6,7: core's out-half wo)
    wcat_d = nc.dram_tensor("wcat", [P, NW, KT, CB], BF16,
                            kind="ExternalInput")
    b_d = nc.dram_tensor("biases", [CB, 10], F32, kind="ExternalInput")
    # out_a: identity epilogue (mu), out_b: softplus epilogue (var);
    # rows [0:128) = col-tile 0, [128:256) = col-tile 1 of the core's
    # 256-wide output block. Host keeps a for cores 0-3, b for 4-7.
    out_a_d = nc.dram_tensor("out_a", [2 * CB, L], F32,
                             kind="ExternalOutput")
    out_b_d = nc.dram_tensor("out_b", [2 * CB, L], F32,
                             kind="ExternalOutput")

    with tile.TileContext(nc) as tc:
        _build(nc, tc, xcat_d, wcat_d, b_d, out_a_d, out_b_d)
    nc.compile()
    return nc


def _build(nc, tc, xcat_d, wcat_d, b_d, out_a_d, out_b_d):
    from contextlib import ExitStack
    ctx = ExitStack()
    with ctx:
        const = ctx.enter_context(tc.tile_pool(name="const", bufs=1))
        persist = ctx.enter_context(tc.tile_pool(name="persist", bufs=1))
        stage = ctx.enter_context(tc.tile_pool(name="stage", bufs=3))
        feat = ctx.enter_context(tc.tile_pool(name="feat", bufs=1))
        attnp = ctx.enter_context(tc.tile_pool(name="attnp", bufs=2))
        ps_proj = ctx.enter_context(
            tc.tile_pool(name="ps_proj", bufs=2, space="PSUM"))
        ps_tr = ctx.enter_context(
            tc.tile_pool(name="ps_tr", bufs=2, space="PSUM"))
        ps_s = ctx.enter_context(
            tc.tile_pool(name="ps_s", bufs=2, space="PSUM"))
        ps_small = ctx.enter_context(
            tc.tile_pool(name="ps_small", bufs=1, space="PSUM"))
        dram = ctx.enter_context(tc.tile_pool(name="dram", bufs=1,
                                              space="DRAM"))

        # ---------------- inputs: clean bf16 DMAs, finest useful order --
        # First projection (wq_var @ x_var) is the PE critical path start:
        # split its two operands into half-kt chunks so it can begin as
        # soon as the first halves land.
        x_sb = persist.tile([P, 2, KT, L], BF16, tag="x_sb", name="x_sb")
        w_sb = persist.tile([P, NW, KT, CB], BF16, tag="w_sb", name="w_sb")
        QK = KT // 4
        nc.sync.dma_start(x_sb[:, 0, 0:QK], xcat_d.ap()[:, 0, 0:QK])
        nc.scalar.dma_start(w_sb[:, 0, 0:QK], wcat_d.ap()[:, 0, 0:QK])
        nc.sync.dma_start(x_sb[:, 0, QK:KT], xcat_d.ap()[:, 0, QK:KT])
        nc.scalar.dma_start(w_sb[:, 0, QK:KT], wcat_d.ap()[:, 0, QK:KT])
        # later weights spread across issue queues so their DIRECT2D
        # descriptor generation runs in parallel and lands them sooner
        nc.gpsimd.dma_start(w_sb[:, 1:2], wcat_d.ap()[:, 1:2])  # k_var
        nc.sync.dma_start(w_sb[:, 2:3], wcat_d.ap()[:, 2:3])    # v_var
        nc.scalar.dma_start(x_sb[:, 1], xcat_d.ap()[:, 1])      # mu
        nc.gpsimd.dma_start(w_sb[:, 3:4], wcat_d.ap()[:, 3:4])  # q_mu
        nc.sync.dma_start(w_sb[:, 4:5], wcat_d.ap()[:, 4:5])    # k_mu
        nc.gpsimd.dma_start(w_sb[:, 5:6], wcat_d.ap()[:, 5:6])  # v_mu
        nc.sync.dma_start(w_sb[:, 6:8], wcat_d.ap()[:, 6:8])    # out-half wo

        # ---------------- constants -----------------------------------
        ident_b = const.tile([P, P], BF16, tag="ident_b", name="ident_b")
        make_identity(nc, ident_b)
        half2 = const.tile([2, P], BF16, tag="half2", name="half2")
        nc.vector.memset(half2, 0.5)
        # ind_h[p, :] = 0.5 if p in head h else 0 (both cols; the fp32 r
        # matmul then yields r_h duplicated on partitions {0,1})
        ind_h = []
        for h in range(2):
            ih = const.tile([P, 2], F32, tag=f"ind{h}", name=f"ind{h}")
            nc.vector.memset(ih, 0.0)
            nc.vector.memset(ih[ts(h, Dh), :], 0.5)
            ind_h.append(ih)

        B_ORDER = ["bq_mu", "bq_var", "bk_mu", "bk_var", "bv_mu", "bv_var",
                   "bo_mu", "bo_var", "bout0", "bout1"]
        bcat = const.tile([CB, 10], F32, tag="bcat", name="bcat")
        nc.sync.dma_start(bcat, b_d.ap())
        nbcat = const.tile([CB, 10], F32, tag="nbcat", name="nbcat")
        nc.vector.tensor_scalar_mul(nbcat, bcat, -1.0)
        bias = {n: bcat[:, i:i + 1] for i, n in enumerate(B_ORDER)}
        nbias = {n: nbcat[:, i:i + 1] for i, n in enumerate(B_ORDER)}

        # warm the sigmoid table while DMAs run
        warm_sg = stage.tile([1, 1], F32, tag="warm_sg", name="warm_sg",
                             bufs=1)
        nc.scalar.activation(warm_sg, ident_b[0:1, 0:1], AF.Sigmoid)

        # ---------------- projections (feature-major [CB, L], bf16) ----
        def project(wi, si):
            ps = ps_proj.tile([P, L], F32, tag="proj", name="proj")
            for kt in range(KT):
                nc.tensor.matmul(ps, w_sb[:, wi, kt, :], x_sb[:, si, kt, :],
                                 start=(kt == 0), stop=(kt == KT - 1))
            return ps

        # var side first: 3 sigmoids (one table residency), then one wide
        # Ln over all three at once (forces sig/sig/sig/ln queue order).
        sg_all = feat.tile([P, 3 * L], F32, tag="sg_all", name="sg_all")
        ps_qv = project(0, 0)
        nc.scalar.activation(sg_all[:, 0 * L:1 * L], ps_qv, AF.Sigmoid,
                             scale=-1.0, bias=nbias["bq_var"])
        ps_kv = project(1, 0)
        nc.scalar.activation(sg_all[:, 1 * L:2 * L], ps_kv, AF.Sigmoid,
                             scale=-1.0, bias=nbias["bk_var"])
        ps_vv = project(2, 0)
        nc.scalar.activation(sg_all[:, 2 * L:3 * L], ps_vv, AF.Sigmoid,
                             scale=-1.0, bias=nbias["bv_var"])

        # nsp_* = ln(sigmoid(-(x+b))) = -softplus(x+b)
        nsp_all = feat.tile([P, 3 * L], F32, tag="nsp_all", name="nsp_all")
        nc.scalar.activation(nsp_all, sg_all, AF.Ln)
        nsp_q = nsp_all[:, 0 * L:1 * L]
        nsp_k = nsp_all[:, 1 * L:2 * L]
        nsp_v = nsp_all[:, 2 * L:3 * L]

        t_kv = feat.tile([P, L], F32, tag="t_kv", name="t_kv")
        nc.vector.tensor_scalar_mul(t_kv, nsp_k, -1.0)    # Kv
        t_lg = feat.tile([P, L], F32, tag="t_lg", name="t_lg")
        nc.scalar.activation(t_lg, t_kv, AF.Ln)           # ln Kv

        # mu-side projections (DVE-only extraction)
        ps_qm = project(3, 1)
        t_qm = feat.tile([P, L], F32, tag="t_qm", name="t_qm")
        nc.vector.tensor_scalar_add(t_qm, ps_qm, bias["bq_mu"])
        negqm_bf = feat.tile([P, L], BF16, tag="negqm", name="negqm")
        nc.vector.tensor_scalar_mul(negqm_bf, t_qm, -1.0)
        t_qm2 = feat.tile([P, L], F32, tag="t_qm2", name="t_qm2")
        nc.vector.tensor_mul(t_qm2, t_qm, t_qm)
        ps_km = project(4, 1)
        t_km = feat.tile([P, L], F32, tag="t_km", name="t_km")
        nc.vector.tensor_scalar_add(t_km, ps_km, bias["bk_mu"])
        t_km2 = feat.tile([P, L], F32, tag="t_km2", name="t_km2")
        nc.vector.tensor_mul(t_km2, t_km, t_km)
        ps_vm = project(5, 1)
        vmT = feat.tile([P, L], BF16, tag="vmT", name="vmT")
        nc.vector.tensor_scalar_add(vmT, ps_vm, bias["bv_mu"])

        # derived features
        fq1_bf = feat.tile([P, L], BF16, tag="fq1", name="fq1")
        nc.vector.tensor_sub(fq1_bf, t_qm2, nsp_q)        # Qm^2 + Qv
        vvT = feat.tile([P, L], BF16, tag="vvT", name="vvT")
        nc.vector.tensor_scalar_mul(vvT, nsp_v, -1.0)     # Vv
        # 1/Kv = exp(-ln Kv) on ACT: its table load starts the moment t_lg
        # finishes (it is the first exp op in the queue, and its t_lg
        # input stops the scheduler hoisting it into the sigmoid block),
        # so the attention exps later hit a resident table. This path is
        # the critical chain into the scores' r-term: keep it short.
        t_iv = feat.tile([P, L], F32, tag="t_iv", name="t_iv")
        nc.scalar.activation(t_iv, t_lg, AF.Exp, scale=-1.0)
        fk1_bf = feat.tile([P, L], BF16, tag="fk1", name="fk1")
        nc.vector.tensor_scalar_mul(fk1_bf, t_iv, 0.5)
        kmiv_bf = feat.tile([P, L], BF16, tag="kmiv", name="kmiv")
        nc.vector.tensor_mul(kmiv_bf, t_km, t_iv)
        t_u = feat.tile([P, L], F32, tag="t_u", name="t_u")
        nc.vector.tensor_mul(t_u, t_km2, t_iv)
        t_s = feat.tile([P, L], F32, tag="t_s", name="t_s")
        nc.vector.tensor_add(t_s, t_u, t_lg)

        # V to L-major bf16 via PE transpose
        v_l = {}
        for nm, src in (("vm", vmT), ("vv", vvT)):
            for lk in range(LT):
                pt = ps_tr.tile([P, P], BF16, tag="tr", name="trb")
                nc.tensor.transpose(pt, src[:, ts(lk, P)], ident_b)
                dst = feat.tile([P, P], BF16, tag=f"vl_{nm}_{lk}",
                                name=f"vl_{nm}_{lk}")
                nc.vector.tensor_copy(dst, pt)
                v_l[(nm, lk)] = dst

        # r per head: 0.5 * sum_d (Km^2/Kv + log Kv) (fp32 path). One
        # duplicate-row matmul per head puts r_h on partitions {0,1};
        # the bf16 hi/lo tiles keep both rows identical (no partition
        # shifts) and are added to the logits via K=2 matmuls against a
        # 0.5-valued lhsT (0.5*(v+v) = v).
        r_hi, r_lo = [], []
        for h in range(2):
            prh = ps_small.tile([2, L], F32, tag="r_ps",
                                name=f"r_ps{h}")
            nc.tensor.matmul(prh, ind_h[h], t_s, start=True, stop=True)
            hi2 = feat.tile([2, L], BF16, tag=f"r_hi{h}", name=f"r_hi{h}")
            nc.vector.tensor_copy(hi2, prh)
            lo2f = feat.tile([2, L], F32, tag=f"r_lof{h}", name=f"r_lof{h}")
            nc.vector.tensor_sub(lo2f, prh, hi2)
            lo2 = feat.tile([2, L], BF16, tag=f"r_lo{h}", name=f"r_lo{h}")
            nc.vector.tensor_copy(lo2, lo2f)
            r_hi.append(hi2)
            r_lo.append(lo2)

        # ---------------- attention ------------------------------------
        # Phase 1 streams all score matmuls + softmax (the in-order PE
        # queue must not hold transposes that wait on the ACT/DVE chain);
        # phase 2 then does all the attn transposes + squares.
        a_bfs = {}
        for h in range(2):
            hs = ts(h, Dh)
            for t in range(LT):
                ps_S = ps_s.tile([P, L], F32, tag="scores", name="scores")
                nc.tensor.matmul(ps_S, fq1_bf[hs, ts(t, P)], fk1_bf[hs, :],
                                 start=True, stop=False)
                nc.tensor.matmul(ps_S, negqm_bf[hs, ts(t, P)], kmiv_bf[hs, :],
                                 start=False, stop=False)
                nc.tensor.matmul(ps_S, half2, r_hi[h],
                                 start=False, stop=False)
                nc.tensor.matmul(ps_S, half2, r_lo[h],
                                 start=False, stop=True)
                pexp = attnp.tile([P, L], BF16, tag="pexp", name="pexp")
                den = attnp.tile([P, 1], F32, tag="den", name="den")
                nc.scalar.activation(pexp, ps_S, AF.Exp, bias=0.0,
                                     scale=-0.125, accum_out=den)
                invd = attnp.tile([P, 1], F32, tag="invd", name="invd")
                nc.vector.reciprocal(invd, den)
                a_bf = attnp.tile([P, L], BF16, tag=f"a_bf_{h}_{t}",
                                  name=f"a_bf_{h}_{t}", bufs=1)
                nc.vector.tensor_scalar_mul(a_bf, pexp, invd)
                a_bfs[(h, t)] = a_bf

        attnT = {}   # (h, lk) -> [128 (k within lk), 256 (q)] bf16
        a2T = {}     # squared attention (var path)
        for h in range(2):
            for t in range(LT):
                for lk in range(LT):
                    if (h, lk) not in attnT:
                        attnT[(h, lk)] = feat.tile(
                            [P, L], BF16, tag=f"attnT_{h}_{lk}",
                            name=f"attnT_{h}_{lk}")
                        a2T[(h, lk)] = feat.tile(
                            [P, L], BF16, tag=f"a2T_{h}_{lk}",
                            name=f"a2T_{h}_{lk}")
                    pt = ps_tr.tile([P, P], BF16, tag="tr", name="trb")
                    nc.tensor.transpose(pt, a_bfs[(h, t)][:, ts(lk, P)],
                                        ident_b)
                    nc.vector.tensor_copy(attnT[(h, lk)][:, ts(t, P)], pt)
                    nc.vector.tensor_mul(a2T[(h, lk)][:, ts(t, P)],
                                         attnT[(h, lk)][:, ts(t, P)],
                                         attnT[(h, lk)][:, ts(t, P)])

        # ---------------- PV + single AllToAll --------------------------
        # cc_in chunk j: our O_mu block for j<4, our O_var block for j>=4
        # => core j receives all 8 cores' blocks of the half IT projects.
        cc_in = dram.tile([NCORES * CB, L], BF16, tag="cc_in", name="cc_in")

        def pv_and_stage(vkey, att, oname, row0, eng):
            # per-head pipelining: head h's PSUM->bf16 copy and its
            # broadcast staging DMA (one DIRECT2D writes all 4 chunk
            # copies) overlap head h+1's PV matmuls
            pv = ps_small.tile([P, L], F32, tag="pv", name=f"pv_{oname}",
                               bufs=1)
            o = attnp.tile([P, L], BF16, tag=f"o_{oname}", name=f"o_{oname}")
            dst = cc_in[row0:row0 + 4 * CB, :].rearrange(
                "(b p) l -> p b l", p=P)
            for h in range(2):
                hs = ts(h, Dh)
                for lk in range(LT):
                    nc.tensor.matmul(pv[hs, :],
                                     v_l[(vkey, lk)][:, hs],
                                     att[(h, lk)],
                                     start=(lk == 0), stop=(lk == LT - 1),
                                     tile_position=(0, h * Dh))
                nc.vector.tensor_copy(o[hs, :], pv[hs, :])
                eng.dma_start(
                    dst[hs],
                    o[hs, :].unsqueeze(1).broadcast_to([Dh, 4, L]))

        pv_and_stage("vm", attnT, "mu", 0, nc.scalar)
        pv_and_stage("vv", a2T, "var", 4 * CB, nc.sync)

        cc_out = dram.tile([NCORES * CB, L], BF16, tag="cc_out",
                           name="cc_out")
        nc.gpsimd.collective_compute(
            "AllToAll", ALU.bypass,
            replica_groups=[list(range(NCORES))],
            ins=[cc_in[:].opt()],
            outs=[cc_out[:].opt()],
        )

        # ---------------- output projection (one half, 256 cols) -------
        g = stage.tile([P, KT, L], BF16, tag="gall", name="gall", bufs=1)
        rr = cc_out.rearrange("(c p) m -> p c m", p=P)
        # 4 chunks on alternating queues: the first out-proj matmuls
        # start after 128KB instead of 512KB
        for ci, eng in ((0, nc.sync), (1, nc.scalar), (2, nc.gpsimd),
                        (3, nc.sync)):
            nc_lo, nc_hi = ci * (KT // 4), (ci + 1) * (KT // 4)
            eng.dma_start(g[:, nc_lo:nc_hi], rr[:, nc_lo:nc_hi])

        # Both epilogues on both col-tiles; u/w1/res are [P, 2L] wide so
        # the single final Ln depends on BOTH exps (no table thrash).
        u = stage.tile([P, 2 * L], F32, tag="u", name="u", bufs=1)
        w1 = stage.tile([P, 2 * L], F32, tag="w1", name="w1", bufs=1)
        res_sp = stage.tile([P, 2 * L], F32, tag="res_sp", name="res_sp",
                            bufs=1)
        ps_o = []
        for m in range(2):
            ps = ps_proj.tile([P, L], F32, tag="proj", name="proj")
            for kt in range(KT):
                nc.tensor.matmul(ps, w_sb[:, 6 + m, kt, :], g[:, kt, :],
                                 start=(kt == 0), stop=(kt == KT - 1))
            nc.scalar.activation(u[:, ts(m, L)], ps, AF.Exp, scale=1.0,
                                 bias=bias["bout0" if m == 0 else "bout1"])
            ps_o.append(ps)
        nc.vector.tensor_scalar_add(w1, u, 1.0)
        nc.scalar.activation(res_sp, w1, AF.Ln)
        for m in range(2):
            res_id = stage.tile([P, L], F32, tag=f"res_id{m}",
                                name=f"res_id{m}")
            nc.vector.tensor_scalar_add(
                res_id, ps_o[m], bias["bout0" if m == 0 else "bout1"])
            nc.gpsimd.dma_start(out_a_d.ap()[ts(m, CB), :], res_id)
            nc.sync.dma_start(out_b_d.ap()[ts(m, CB), :], res_sp[:, ts(m, L)])


def shard_inputs(inputs):
    """Full inputs -> per-core in_maps (host-side numpy prep only)."""
    f32 = np.float32
    bf16 = mybir.dt.np(BF16)

    def to_pe_tiles(a):      # [1024, n] -> [128, 8, n]
        n = a.shape[1]
        return np.ascontiguousarray(
            a.reshape(KT, P, n).transpose(1, 0, 2))

    xcat = np.empty((P, 2, KT, L), dtype=bf16)
    for si, nm in enumerate(("var", "mu")):
        xt = np.asarray(inputs[nm]).reshape(L, D).astype(f32).T  # [D, L]
        xcat[:, si] = to_pe_tiles(xt.astype(bf16))

    W_ORDER = ["wq_var", "wk_var", "wv_var", "wq_mu", "wk_mu", "wv_mu"]
    B_NAMES = ["bq_mu", "bq_var", "bk_mu", "bk_var", "bv_mu", "bv_var",
               "bo_mu", "bo_var"]
    in_maps = []
    for c in range(NCORES):
        cols = slice(c * CB, (c + 1) * CB)
        wcat = np.empty((P, NW, KT, CB), dtype=bf16)
        for wi, nm in enumerate(W_ORDER):
            w = np.asarray(inputs[nm])[:, cols].astype(f32).astype(bf16)
            wcat[:, wi] = to_pe_tiles(w)
        # out-half role: cores 0-3 -> wo_mu/bo_mu cols [256c, 256c+256),
        # cores 4-7 -> wo_var/bo_var cols [256(c-4), ...)
        if c < 4:
            wo, bo, oc = inputs["wo_mu"], inputs["bo_mu"], 2 * c * CB
        else:
            wo, bo, oc = inputs["wo_var"], inputs["bo_var"], 2 * (c - 4) * CB
        for m in range(2):
            wblk = np.asarray(wo)[:, oc + m * CB: oc + (m + 1) * CB]
            wcat[:, 6 + m] = to_pe_tiles(wblk.astype(f32).astype(bf16))
        bcols = [np.asarray(inputs[n])[cols].astype(f32) for n in B_NAMES]
        bcols.append(np.asarray(bo)[oc: oc + CB].astype(f32))
        bcols.append(np.asarray(bo)[oc + CB: oc + 2 * CB].astype(f32))
        biases = np.ascontiguousarray(np.stack(bcols, axis=1))
        in_maps.append({"xcat": xcat, "wcat": wcat, "biases": biases})
    return in_maps


def kernel(**inputs):
    global LAST_RESULT
    if "prog" not in _prog_cache:
        _prog_cache["prog"] = build_program()
    nc = _prog_cache["prog"]
    in_maps = shard_inputs(inputs)
    res = run_bass_kernel_spmd(nc, in_maps, core_ids=list(range(NCORES)),
                               trace=TRACE, **TRACE_KWARGS)
    LAST_RESULT = res
    mu_blocks = [res.results[c]["out_a"] for c in range(4)]
    var_blocks = [res.results[c]["out_b"] for c in range(4, 8)]
    mu_out = np.concatenate(mu_blocks, axis=0).T.reshape(B, L, D)
    var_out = np.concatenate(var_blocks, axis=0).T.reshape(B, L, D)
    return (np.ascontiguousarray(mu_out.astype(np.float32)),
            np.ascontiguousarray(var_out.astype(np.float32)))


# revision 46
# speedup vs baseline: 1.9146x; 1.0633x over previous
"""Distributional (Gaussian-KL) attention on 8 TRN2 NeuronCores.

Math: for each head, the KL-based score decomposes as
    kl[q,k] = sum_d (Qm-Km)^2/(2Kv) + 0.5*(Qv/Kv - log(Qv/Kv) - 1)
            = Fq[q] . Fk[k] + r[k] + c[q]
with  Fq = [Qm^2+Qv ; -Qm],  Fk = [1/(2Kv) ; Km/Kv],
      r  = 0.5*sum_d (Km^2/Kv + log Kv),
and c[q] only shifts softmax logits per-row (drops out of softmax over k).
scores = -kl/sqrt(Dh); kl >= 0 so exp(scores) needs no max-shift.

Sharding: head-parallel front (core c owns heads {2c,2c+1} == feature
columns [128c,128c+128) of every Q/K/V projection), then SPLIT-ROLE
output: cores 0-3 compute out_mu columns [256c,256c+256), cores 4-7
compute out_var columns [256(c-4),...). The role is purely data-driven
(per-core wo/bias inputs), so the program stays SPMD. A single AllToAll
delivers to each core the full O-half it needs (chunk j of core i's send
buffer = i's O_mu block for j<4, else its O_var block): one collective,
half the inbound bytes of a full AllGather of both halves. Every core
runs BOTH epilogues (identity for mu, softplus for var) on its projected
block; the host keeps the meaningful one per core.

Host-side prep (inside kernel(), numpy only): weights/inputs pre-cast to
bf16 and pre-transposed/tiled into PE-friendly layouts - no on-device
casts or input transposes.

Precision: all matmuls bf16 except the r_k path (fp32; r is O(100+) and
added to logits as a bf16 hi+lo pair via K=2 rank-2 matmuls against a
0.5-valued lhsT, since duplicate rows avoid illegal partition bases).

ACT table discipline (a table swap costs ~1.28us): sigmoid x3 -> one
wide Ln (+ t_lg Ln) -> exp (attention + output-softplus numerator; a
dep-free warm op prefetches the table during the score matmuls) -> one
wide final Ln.  softplus(x)=ln(sigmoid(-x)) up front, ln(1+exp(x)) at
the output so it reuses the resident exp table.  1/Kv runs on DVE in
parallel with ACT.
"""

import numpy as np

import concourse.bass as bass
import concourse.mybir as mybir
import concourse.tile as tile
from concourse import bacc
from concourse.masks import make_identity
from concourse.bass_utils import run_bass_kernel_spmd

F32 = mybir.dt.float32
BF16 = mybir.dt.bfloat16
AF = mybir.ActivationFunctionType
ALU = mybir.AluOpType
AX = mybir.AxisListType

H, B, L, D = 16, 1, 256, 1024
Dh = D // H          # 64
NCORES = 8
CB = D // NCORES     # 128 feature columns per core (2 heads)
P = 128
LT = L // P          # 2 row tiles of the sequence
KT = D // P          # 8 contraction tiles
NW = 8               # weights: qv kv vv qm km vm | out-half col-tiles 0,1

TRACE = False
TRACE_KWARGS = {}
LAST_RESULT = None

_prog_cache = {}


def ts(i, size):
    return slice(i * size, (i + 1) * size)


def build_program():
    nc = bacc.Bacc("TRN2", target_bir_lowering=False, debug=False,
                   num_devices=NCORES)

    # xcat[p, s, kt, l] = x_s[l, kt*128+p]; s: 0=var, 1=mu
    xcat_d = nc.dram_tensor("xcat", [P, 2, KT, L], BF16, kind="ExternalInput")
    # wcat[p, w, kt, m] = w[kt*128+p, m] (slots 6,7: core's out-half wo)
    wcat_d = nc.dram_tensor("wcat", [P, NW, KT, CB], BF16,
                            kind="ExternalInput")
    b_d = nc.dram_tensor("biases", [CB, 10], F32, kind="ExternalInput")
    # out_a: identity epilogue (mu), out_b: softplus epilogue (var);
    # rows [0:128) = col-tile 0, [128:256) = col-tile 1 of the core's
    # 256-wide output block. Host keeps a for cores 0-3, b for 4-7.
    out_a_d = nc.dram_tensor("out_a", [2 * CB, L], F32,
                             kind="ExternalOutput")
    out_b_d = nc.dram_tensor("out_b", [2 * CB, L], F32,
                             kind="ExternalOutput")

    with tile.TileContext(nc) as tc:
        _build(nc, tc, xcat_d, wcat_d, b_d, out_a_d, out_b_d)
    nc.compile()
    return nc


def _build(nc, tc, xcat_d, wcat_d, b_d, out_a_d, out_b_d):
    from contextlib import ExitStack
    ctx = ExitStack()
    with ctx:
        const = ctx.enter_context(tc.tile_pool(name="const", bufs=1))
        persist = ctx.enter_context(tc.tile_pool(name="persist", bufs=1))
        stage = ctx.enter_context(tc.tile_pool(name="stage", bufs=3))
        feat = ctx.enter_context(tc.tile_pool(name="feat", bufs=1))
        attnp = ctx.enter_context(tc.tile_pool(name="attnp", bufs=2))
        ps_proj = ctx.enter_context(
            tc.tile_pool(name="ps_proj", bufs=2, space="PSUM"))
        ps_tr = ctx.enter_context(
            tc.tile_pool(name="ps_tr", bufs=2, space="PSUM"))
        ps_s = ctx.enter_context(
            tc.tile_pool(name="ps_s", bufs=2, space="PSUM"))
        ps_small = ctx.enter_context(
            tc.tile_pool(name="ps_small", bufs=1, space="PSUM"))
        dram = ctx.enter_context(tc.tile_pool(name="dram", bufs=1,
                                              space="DRAM"))

        # ---------------- inputs: clean bf16 DMAs, finest useful order --
        # First projection (wq_var @ x_var) is the PE critical path start:
        # split its two operands into half-kt chunks so it can begin as
        # soon as the first halves land.
        x_sb = persist.tile([P, 2, KT, L], BF16, tag="x_sb", name="x_sb")
        w_sb = persist.tile([P, NW, KT, CB], BF16, tag="w_sb", name="w_sb")
        QK = KT // 4
        nc.sync.dma_start(x_sb[:, 0, 0:QK], xcat_d.ap()[:, 0, 0:QK])
        nc.scalar.dma_start(w_sb[:, 0, 0:QK], wcat_d.ap()[:, 0, 0:QK])
        nc.sync.dma_start(x_sb[:, 0, QK:KT], xcat_d.ap()[:, 0, QK:KT])
        nc.scalar.dma_start(w_sb[:, 0, QK:KT], wcat_d.ap()[:, 0, QK:KT])
        # later weights spread across issue queues so their DIRECT2D
        # descriptor generation runs in parallel and lands them sooner
        nc.gpsimd.dma_start(w_sb[:, 1:2], wcat_d.ap()[:, 1:2])  # k_var
        nc.sync.dma_start(w_sb[:, 2:3], wcat_d.ap()[:, 2:3])    # v_var
        nc.scalar.dma_start(x_sb[:, 1], xcat_d.ap()[:, 1])      # mu
        nc.gpsimd.dma_start(w_sb[:, 3:4], wcat_d.ap()[:, 3:4])  # q_mu
        nc.sync.dma_start(w_sb[:, 4:5], wcat_d.ap()[:, 4:5])    # k_mu
        nc.gpsimd.dma_start(w_sb[:, 5:6], wcat_d.ap()[:, 5:6])  # v_mu
        nc.sync.dma_start(w_sb[:, 6:8], wcat_d.ap()[:, 6:8])    # out-half wo

        # ---------------- constants -----------------------------------
        ident_b = const.tile([P, P], BF16, tag="ident_b", name="ident_b")
        make_identity(nc, ident_b)
        half2 = const.tile([2, P], BF16, tag="half2", name="half2")
        nc.vector.memset(half2, 0.5)
        # ind_h[p, :] = 0.5 if p in head h else 0 (both cols; the fp32 r
        # matmul then yields r_h duplicated on partitions {0,1})
        ind_h = []
        for h in range(2):
            ih = const.tile([P, 2], F32, tag=f"ind{h}", name=f"ind{h}")
            nc.vector.memset(ih, 0.0)
            nc.vector.memset(ih[ts(h, Dh), :], 0.5)
            ind_h.append(ih)

        B_ORDER = ["bq_mu", "bq_var", "bk_mu", "bk_var", "bv_mu", "bv_var",
                   "bo_mu", "bo_var", "bout0", "bout1"]
        bcat = const.tile([CB, 10], F32, tag="bcat", name="bcat")
        nc.sync.dma_start(bcat, b_d.ap())
        nbcat = const.tile([CB, 10], F32, tag="nbcat", name="nbcat")
        nc.vector.tensor_scalar_mul(nbcat, bcat, -1.0)
        bias = {n: bcat[:, i:i + 1] for i, n in enumerate(B_ORDER)}
        nbias = {n: nbcat[:, i:i + 1] for i, n in enumerate(B_ORDER)}

        # warm the sigmoid table while DMAs run
        warm_sg = stage.tile([1, 1], F32, tag="warm_sg", name="warm_sg",
                             bufs=1)
        nc.scalar.activation(warm_sg, ident_b[0:1, 0:1], AF.Sigmoid)

        # ---------------- projections (feature-major [CB, L], bf16) ----
        def project(wi, si):
            ps = ps_proj.tile([P, L], F32, tag="proj", name="proj")
            for kt in range(KT):
                nc.tensor.matmul(ps, w_sb[:, wi, kt, :], x_sb[:, si, kt, :],
                                 start=(kt == 0), stop=(kt == KT - 1))
            return ps

        # var side first: 3 sigmoids (one table residency), then one wide
        # Ln over all three at once (forces sig/sig/sig/ln queue order).
        sg_all = feat.tile([P, 3 * L], F32, tag="sg_all", name="sg_all")
        ps_qv = project(0, 0)
        nc.scalar.activation(sg_all[:, 0 * L:1 * L], ps_qv, AF.Sigmoid,
                             scale=-1.0, bias=nbias["bq_var"])
        ps_kv = project(1, 0)
        nc.scalar.activation(sg_all[:, 1 * L:2 * L], ps_kv, AF.Sigmoid,
                             scale=-1.0, bias=nbias["bk_var"])
        ps_vv = project(2, 0)
        nc.scalar.activation(sg_all[:, 2 * L:3 * L], ps_vv, AF.Sigmoid,
                             scale=-1.0, bias=nbias["bv_var"])

        # nsp_* = ln(sigmoid(-(x+b))) = -softplus(x+b)
        nsp_all = feat.tile([P, 3 * L], F32, tag="nsp_all", name="nsp_all")
        nc.scalar.activation(nsp_all, sg_all, AF.Ln)
        nsp_q = nsp_all[:, 0 * L:1 * L]
        nsp_k = nsp_all[:, 1 * L:2 * L]
        nsp_v = nsp_all[:, 2 * L:3 * L]

        t_kv = feat.tile([P, L], F32, tag="t_kv", name="t_kv")
        nc.vector.tensor_scalar_mul(t_kv, nsp_k, -1.0)    # Kv
        t_lg = feat.tile([P, L], F32, tag="t_lg", name="t_lg")
        nc.scalar.activation(t_lg, t_kv, AF.Ln)           # ln Kv

        # mu-side projections (DVE-only extraction)
        ps_qm = project(3, 1)
        t_qm = feat.tile([P, L], F32, tag="t_qm", name="t_qm")
        nc.vector.tensor_scalar_add(t_qm, ps_qm, bias["bq_mu"])
        negqm_bf = feat.tile([P, L], BF16, tag="negqm", name="negqm")
        nc.vector.tensor_scalar_mul(negqm_bf, t_qm, -1.0)
        t_qm2 = feat.tile([P, L], F32, tag="t_qm2", name="t_qm2")
        nc.vector.tensor_mul(t_qm2, t_qm, t_qm)
        ps_km = project(4, 1)
        t_km = feat.tile([P, L], F32, tag="t_km", name="t_km")
        nc.vector.tensor_scalar_add(t_km, ps_km, bias["bk_mu"])
        t_km2 = feat.tile([P, L], F32, tag="t_km2", name="t_km2")
        nc.vector.tensor_mul(t_km2, t_km, t_km)
        ps_vm = project(5, 1)
        vmT = feat.tile([P, L], BF16, tag="vmT", name="vmT")
        nc.vector.tensor_scalar_add(vmT, ps_vm, bias["bv_mu"])

        # derived features
        fq1_bf = feat.tile([P, L], BF16, tag="fq1", name="fq1")
        nc.vector.tensor_sub(fq1_bf, t_qm2, nsp_q)        # Qm^2 + Qv
        vvT = feat.tile([P, L], BF16, tag="vvT", name="vvT")
        nc.vector.tensor_scalar_mul(vvT, nsp_v, -1.0)     # Vv
        # 1/Kv = exp(-ln Kv) on ACT: its table load starts the moment t_lg
        # finishes (it is the first exp op in the queue, and its t_lg
        # input stops the scheduler hoisting it into the sigmoid block),
        # so the attention exps later hit a resident table. This path is
        # the critical chain into the scores' r-term: keep it short.
        t_iv = feat.tile([P, L], F32, tag="t_iv", name="t_iv")
        nc.scalar.activation(t_iv, t_lg, AF.Exp, scale=-1.0)
        fk1_bf = feat.tile([P, L], BF16, tag="fk1", name="fk1")
        nc.vector.tensor_scalar_mul(fk1_bf, t_iv, 0.5)
        kmiv_bf = feat.tile([P, L], BF16, tag="kmiv", name="kmiv")
        nc.vector.tensor_mul(kmiv_bf, t_km, t_iv)
        t_u = feat.tile([P, L], F32, tag="t_u", name="t_u")
        nc.vector.tensor_mul(t_u, t_km2, t_iv)
        t_s = feat.tile([P, L], F32, tag="t_s", name="t_s")
        nc.vector.tensor_add(t_s, t_u, t_lg)

        # V to L-major bf16 via PE transpose
        v_l = {}
        for nm, src in (("vm", vmT), ("vv", vvT)):
            for lk in range(LT):
                pt = ps_tr.tile([P, P], BF16, tag="tr", name="trb")
                nc.tensor.transpose(pt, src[:, ts(lk, P)], ident_b)
                dst = feat.tile([P, P], BF16, tag=f"vl_{nm}_{lk}",
                                name=f"vl_{nm}_{lk}")
                nc.vector.tensor_copy(dst, pt)
                v_l[(nm, lk)] = dst

        # r per head: 0.5 * sum_d (Km^2/Kv + log Kv) (fp32 path). One
        # duplicate-row matmul per head puts r_h on partitions {0,1};
        # the bf16 hi/lo tiles keep both rows identical (no partition
        # shifts) and are added to the logits via K=2 matmuls against a
        # 0.5-valued lhsT (0.5*(v+v) = v).
        r_hi, r_lo = [], []
        for h in range(2):
            prh = ps_small.tile([2, L], F32, tag="r_ps",
                                name=f"r_ps{h}")
            nc.tensor.matmul(prh, ind_h[h], t_s, start=True, stop=True)
            hi2 = feat.tile([2, L], BF16, tag=f"r_hi{h}", name=f"r_hi{h}")
            nc.vector.tensor_copy(hi2, prh)
            lo2f = feat.tile([2, L], F32, tag=f"r_lof{h}", name=f"r_lof{h}")
            nc.vector.tensor_sub(lo2f, prh, hi2)
            lo2 = feat.tile([2, L], BF16, tag=f"r_lo{h}", name=f"r_lo{h}")
            nc.vector.tensor_copy(lo2, lo2f)
            r_hi.append(hi2)
            r_lo.append(lo2)

        # ---------------- attention ------------------------------------
        # Phase 1 streams all score matmuls + softmax (the in-order PE
        # queue must not hold transposes that wait on the ACT/DVE chain);
        # phase 2 then does all the attn transposes + squares.
        a_bfs = {}
        for h in range(2):
            hs = ts(h, Dh)
            for t in range(LT):
                ps_S = ps_s.tile([P, L], F32, tag="scores", name="scores")
                nc.tensor.matmul(ps_S, fq1_bf[hs, ts(t, P)], fk1_bf[hs, :],
                                 start=True, stop=False)
                nc.tensor.matmul(ps_S, negqm_bf[hs, ts(t, P)], kmiv_bf[hs, :],
                                 start=False, stop=False)
                nc.tensor.matmul(ps_S, half2, r_hi[h],
                                 start=False, stop=False)
                nc.tensor.matmul(ps_S, half2, r_lo[h],
                                 start=False, stop=True)
                pexp = attnp.tile([P, L], BF16, tag="pexp", name="pexp")
                den = attnp.tile([P, 1], F32, tag="den", name="den")
                nc.scalar.activation(pexp, ps_S, AF.Exp, bias=0.0,
                                     scale=-0.125, accum_out=den)
                invd = attnp.tile([P, 1], F32, tag="invd", name="invd")
                nc.vector.reciprocal(invd, den)
                a_bf = attnp.tile([P, L], BF16, tag=f"a_bf_{h}_{t}",
                                  name=f"a_bf_{h}_{t}", bufs=1)
                nc.vector.tensor_scalar_mul(a_bf, pexp, invd)
                a_bfs[(h, t)] = a_bf

        attnT = {}   # (h, lk) -> [128 (k within lk), 256 (q)] bf16
        a2T = {}     # squared attention (var path)
        for h in range(2):
            for t in range(LT):
                for lk in range(LT):
                    if (h, lk) not in attnT:
                        attnT[(h, lk)] = feat.tile(
                            [P, L], BF16, tag=f"attnT_{h}_{lk}",
                            name=f"attnT_{h}_{lk}")
                        a2T[(h, lk)] = feat.tile(
                            [P, L], BF16, tag=f"a2T_{h}_{lk}",
                            name=f"a2T_{h}_{lk}")
                    pt = ps_tr.tile([P, P], BF16, tag="tr", name="trb")
                    nc.tensor.transpose(pt, a_bfs[(h, t)][:, ts(lk, P)],
                                        ident_b)
                    nc.vector.tensor_copy(attnT[(h, lk)][:, ts(t, P)], pt)
                    nc.vector.tensor_mul(a2T[(h, lk)][:, ts(t, P)],
                                         attnT[(h, lk)][:, ts(t, P)],
                                         attnT[(h, lk)][:, ts(t, P)])

        # ---------------- PV + single AllToAll --------------------------
        # cc_in chunk j: our O_mu block for j<4, our O_var block for j>=4
        # => core j receives all 8 cores' blocks of the half IT projects.
        cc_in = dram.tile([NCORES * CB, L], BF16, tag="cc_in", name="cc_in")

        def pv_and_stage(vkey, att, oname, row0, engs):
            # per-head pipelining: head h's PSUM->bf16 copy and its
            # broadcast staging DMA (one DIRECT2D writes all 4 chunk
            # copies) overlap head h+1's PV matmuls; the two heads'
            # DIRECT2Ds issue on different queues so they don't
            # serialize on one sequencer right before the trigger
            pv = ps_small.tile([P, L], F32, tag="pv", name=f"pv_{oname}",
                               bufs=1)
            o = attnp.tile([P, L], BF16, tag=f"o_{oname}", name=f"o_{oname}")
            dst = cc_in[row0:row0 + 4 * CB, :].rearrange(
                "(b p) l -> p b l", p=P)
            for h in range(2):
                hs = ts(h, Dh)
                for lk in range(LT):
                    nc.tensor.matmul(pv[hs, :],
                                     v_l[(vkey, lk)][:, hs],
                                     att[(h, lk)],
                                     start=(lk == 0), stop=(lk == LT - 1),
                                     tile_position=(0, h * Dh))
                nc.vector.tensor_copy(o[hs, :], pv[hs, :])
                engs[h].dma_start(
                    dst[hs],
                    o[hs, :].unsqueeze(1).broadcast_to([Dh, 4, L]))

        pv_and_stage("vm", attnT, "mu", 0, (nc.scalar, nc.gpsimd))
        pv_and_stage("vv", a2T, "var", 4 * CB, (nc.sync, nc.scalar))

        cc_out = dram.tile([NCORES * CB, L], BF16, tag="cc_out",
                           name="cc_out")
        nc.gpsimd.collective_compute(
            "AllToAll", ALU.bypass,
            replica_groups=[list(range(NCORES))],
            ins=[cc_in[:].opt()],
            outs=[cc_out[:].opt()],
        )

        # ---------------- output projection (one half, 256 cols) -------
        g = stage.tile([P, KT, L], BF16, tag="gall", name="gall", bufs=1)
        rr = cc_out.rearrange("(c p) m -> p c m", p=P)
        # 4 chunks on alternating queues: the first out-proj matmuls
        # start after 128KB instead of 512KB
        for ci, eng in ((0, nc.sync), (1, nc.scalar), (2, nc.gpsimd),
                        (3, nc.sync)):
            nc_lo, nc_hi = ci * (KT // 4), (ci + 1) * (KT // 4)
            eng.dma_start(g[:, nc_lo:nc_hi], rr[:, nc_lo:nc_hi])

        # Both epilogues on both col-tiles; u/w1/res are [P, 2L] wide so
        # the single final Ln depends on BOTH exps (no table thrash).
        u = stage.tile([P, 2 * L], F32, tag="u", name="u", bufs=1)
        w1 = stage.tile([P, 2 * L], F32, tag="w1", name="w1", bufs=1)
        res_sp = stage.tile([P, 2 * L], F32, tag="res_sp", name="res_sp",
                            bufs=1)
        ps_o = []
        for m in range(2):
            ps = ps_proj.tile([P, L], F32, tag="proj", name="proj")
            for kt in range(KT):
                nc.tensor.matmul(ps, w_sb[:, 6 + m, kt, :], g[:, kt, :],
                                 start=(kt == 0), stop=(kt == KT - 1))
            nc.scalar.activation(u[:, ts(m, L)], ps, AF.Exp, scale=1.0,
                                 bias=bias["bout0" if m == 0 else "bout1"])
            ps_o.append(ps)
        nc.vector.tensor_scalar_add(w1, u, 1.0)
        nc.scalar.activation(res_sp, w1, AF.Ln)
        for m in range(2):
            res_id = stage.tile([P, L], F32, tag=f"res_id{m}",
                                name=f"res_id{m}")
            nc.vector.tensor_scalar_add(
                res_id, ps_o[m], bias["bout0" if m == 0 else "bout1"])
            nc.gpsimd.dma_start(out_a_d.ap()[ts(m, CB), :], res_id)
            nc.sync.dma_start(out_b_d.ap()[ts(m, CB), :], res_sp[:, ts(m, L)])


def shard_inputs(inputs):
    """Full inputs -> per-core in_maps (host-side numpy prep only)."""
    f32 = np.float32
    bf16 = mybir.dt.np(BF16)

    def to_pe_tiles(a):      # [1024, n] -> [128, 8, n]
        n = a.shape[1]
        return np.ascontiguousarray(
            a.reshape(KT, P, n).transpose(1, 0, 2))

    xcat = np.empty((P, 2, KT, L), dtype=bf16)
    for si, nm in enumerate(("var", "mu")):
        xt = np.asarray(inputs[nm]).reshape(L, D).astype(f32).T  # [D, L]
        xcat[:, si] = to_pe_tiles(xt.astype(bf16))

    W_ORDER = ["wq_var", "wk_var", "wv_var", "wq_mu", "wk_mu", "wv_mu"]
    B_NAMES = ["bq_mu", "bq_var", "bk_mu", "bk_var", "bv_mu", "bv_var",
               "bo_mu", "bo_var"]
    in_maps = []
    for c in range(NCORES):
        cols = slice(c * CB, (c + 1) * CB)
        wcat = np.empty((P, NW, KT, CB), dtype=bf16)
        for wi, nm in enumerate(W_ORDER):
            w = np.asarray(inputs[nm])[:, cols].astype(f32).astype(bf16)
            wcat[:, wi] = to_pe_tiles(w)
        # out-half role: cores 0-3 -> wo_mu/bo_mu cols [256c, 256c+256),
        # cores 4-7 -> wo_var/bo_var cols [256(c-4), ...)
        if c < 4:
            wo, bo, oc = inputs["wo_mu"], inputs["bo_mu"], 2 * c * CB
        else:
            wo, bo, oc = inputs["wo_var"], inputs["bo_var"], 2 * (c - 4) * CB
        for m in range(2):
            wblk = np.asarray(wo)[:, oc + m * CB: oc + (m + 1) * CB]
            wcat[:, 6 + m] = to_pe_tiles(wblk.astype(f32).astype(bf16))
        bcols = [np.asarray(inputs[n])[cols].astype(f32) for n in B_NAMES]
        bcols.append(np.asarray(bo)[oc: oc + CB].astype(f32))
        bcols.append(np.asarray(bo)[oc + CB: oc + 2 * CB].astype(f32))
        biases = np.ascontiguousarray(np.stack(bcols, axis=1))
        in_maps.append({"xcat": xcat, "wcat": wcat, "biases": biases})
    return in_maps


def kernel(**inputs):
    global LAST_RESULT
    if "prog" not in _prog_cache:
        _prog_cache["prog"] = build_program()
    nc = _prog_cache["prog"]
    in_maps = shard_inputs(inputs)
    res = run_bass_kernel_spmd(nc, in_maps, core_ids=list(range(NCORES)),
                               trace=TRACE, **TRACE_KWARGS)
    LAST_RESULT = res
    mu_blocks = [res.results[c]["out_a"] for c in range(4)]
    var_blocks = [res.results[c]["out_b"] for c in range(4, 8)]
    mu_out = np.concatenate(mu_blocks, axis=0).T.reshape(B, L, D)
    var_out = np.concatenate(var_blocks, axis=0).T.reshape(B, L, D)
    return (np.ascontiguousarray(mu_out.astype(np.float32)),
            np.ascontiguousarray(var_out.astype(np.float32)))
